# revision 1
# baseline (speedup 1.0000x reference)
"""Trainium2 Bass kernel for the 6-layer BigramLanguageModel (B=2, T=1024,
C=1024, H=16, FFN=4096, V=32000).

Strategy: context-parallel over 8 NeuronCores. Cores 0-3 handle batch 0,
cores 4-7 batch 1; each core owns a contiguous 256-token chunk. Per layer
each core computes LN/QKV/attention/proj/FFN for its own tokens; the only
communication is a 4-rank AllGather of K/V per layer (fp16, split in two
feature-halves so attention on the first half overlaps the gather of the
second) plus one final 8-rank AllGather of the pre-lm_head hidden states.
The lm_head is vocab-sharded 8 ways, computed vocab-major (lhsT = W_lm
tiles, M=125) so all DMA stays large-descriptor; the host transposes the
per-core [4000, 2048] result slices back.

Activations are feature-major ([C, tokens]); LayerNorm reductions over
features (= partitions) use ones-matmuls on the tensor engine, and LN
scale/bias are folded into the consuming weights on the host. Main matmuls
run in float32r (single-pass fp32); the attention QK/PV path runs in fp16
(10-bit mantissa, ample range for unit-scale K/Q/V and softmax weights).
The residual stream stays in fp32. Softmax skips the max-subtraction
(scores are O(1) by construction); the causal mask is per-core input data;
the denominator comes from augmenting V with a ones column (M=65 PV
matmuls).
"""

import os

import numpy as np

# model dims (fixed by the problem)
B, T, V, C, H, HS, L, F = 2, 1024, 32000, 1024, 16, 64, 6, 4096
P = 128
NCORES = 8
GROUP = 4            # cores per batch (context-parallel degree)
NT = T // GROUP      # 256 tokens per core
CCH = C // P         # 8 feature chunks
FCH = F // P         # 32 FFN hidden chunks
KB = T // P          # 8 key blocks
VSH = V // NCORES    # 4000 vocab columns per core
MLM = 32             # lm_head output-partition chunks
MV = VSH // MLM      # 125 vocab rows per chunk
NTK = 4              # lm_head token chunks
TKW = (NCORES * NT) // NTK   # 512 tokens per chunk
EPS = 1e-5
NEG = -1e9

_CACHE = {}


def _build():
    import concourse.bass as bass
    import concourse.tile as tile
    from concourse import bacc, mybir

    f32 = mybir.dt.float32
    f32r = mybir.dt.float32r
    f16 = mybir.dt.float16
    AFT = mybir.ActivationFunctionType
    ALU = mybir.AluOpType

    nc = bacc.Bacc("TRN2", target_bir_lowering=False, debug=False,
                   num_devices=NCORES)

    def din(name, shape, dt=f32r):
        return nc.dram_tensor(name, shape, dt, kind="ExternalInput").ap()

    d_x0 = din("x0t", [C, NT], f32)
    d_wq = din("wq", [L, CCH, P, CCH, P])
    d_wk = din("wk", [L, CCH, P, CCH, P])
    d_wo = din("wo", [L, CCH, P, CCH, P])
    d_w1 = din("w1", [L, FCH, P, CCH, P])
    d_w2 = din("w2", [L, CCH, P, FCH, P])
    d_wv = din("wv", [L, C, C])
    d_wlm = din("wlm", [MLM, P, CCH, MV])
    d_qb = din("qb", [L, P, CCH], f32)
    d_kb = din("kb", [L, P, CCH], f32)
    d_b1 = din("b1", [L, P, FCH], f32)
    d_bo = din("bo", [L, C])
    d_b2 = din("b2", [L, C])
    d_vb = din("vb", [L, C], f32)
    d_blm = din("blm", [P, MLM], f32)
    d_msk = din("msk", [KB, P, NT], f32)
    d_onr = din("onr", [P, NT])
    d_onf = din("onf", [P, 1], f32)

    d_out = nc.dram_tensor("logits", [VSH, NCORES * NT], f32,
                           kind="ExternalOutput").ap()

    groups4 = [[0, 1, 2, 3], [4, 5, 6, 7]]
    groups8 = [list(range(NCORES))]

    with tile.TileContext(nc) as tc:
        with tc.tile_pool(name="persist", bufs=1) as pp, \
             tc.tile_pool(name="act", bufs=1) as ap_, \
             tc.tile_pool(name="wp", bufs=3) as wp, \
             tc.tile_pool(name="sm", bufs=2) as smp, \
             tc.tile_pool(name="ps", bufs=2, space="PSUM") as psp, \
             tc.tile_pool(name="dram", bufs=2, space="DRAM") as dp:

            # ---------------- persistent tiles ----------------
            x = pp.tile([P, CCH, NT], f32)          # residual stream
            msk = pp.tile([P, KB, NT], f32)
            onr = pp.tile([P, NT], f32r)
            onf = pp.tile([P, 1], f32)
            eps_t = pp.tile([1, 1], f32)
            blm_sb = pp.tile([P, MLM], f32)

            nc.sync.dma_start(x[:], d_x0.rearrange("(c p) t -> p c t", p=P))
            nc.sync.dma_start(msk[:], d_msk.rearrange("k p t -> p k t"))
            nc.sync.dma_start(onr[:], d_onr[:])
            nc.sync.dma_start(onf[:], d_onf[:])
            nc.sync.dma_start(blm_sb[:], d_blm[:])
            nc.vector.memset(eps_t[:], EPS)

            def layernorm(src, dst):
                """dst (f32r) = (src - mu) / sqrt(var + eps), feature-major."""
                ps_sx = psp.tile([1, NT], f32, name="ps_sx", tag="ps")
                ps_sq = psp.tile([1, NT], f32, name="ps_sq", tag="ps")
                for c in range(CCH):
                    sq = smp.tile([P, NT], f32, name="sq", tag="sq")
                    nc.scalar.activation(sq[:], src[:, c, :], AFT.Square)
                    nc.tensor.matmul(ps_sx[:], onf[:], src[:, c, :],
                                     start=(c == 0), stop=(c == CCH - 1))
                    nc.tensor.matmul(ps_sq[:], onf[:], sq[:],
                                     start=(c == 0), stop=(c == CCH - 1))
                st = smp.tile([1, 4, NT], f32, name="st_s", tag="st_s")
                mu = st[:, 0, :]
                ex2 = st[:, 1, :]
                var = st[:, 2, :]
                sd = st[:, 3, :]
                nc.scalar.mul(mu, ps_sx[:], 1.0 / C)
                nc.scalar.mul(ex2, ps_sq[:], 1.0 / C)
                nc.vector.tensor_mul(var, mu, mu)
                nc.vector.tensor_sub(var, ex2, var)
                nc.scalar.activation(sd, var, AFT.Sqrt, bias=eps_t[:])
                rsrc = smp.tile([1, 2 * NT], f32r, name="rsrc", tag="rsrc")
                with nc.allow_low_precision(reason="LN rstd in f32r"):
                    nc.vector.reciprocal(rsrc[:, 0:NT], sd)
                    nc.vector.tensor_mul(rsrc[:, NT:2 * NT], mu, rsrc[:, 0:NT])
                ps_rep = psp.tile([P, 2 * NT], f32, name="ps_rep", tag="mm",
                                  bufs=3)
                nc.tensor.matmul(ps_rep[:], onr[0:1, 0:P], rsrc[:],
                                 start=True, stop=True)
                rep = smp.tile([P, 2 * NT], f32, name="rep", tag="rep")
                nc.scalar.copy(rep[:], ps_rep[:])
                for c in range(CCH):
                    t1 = smp.tile([P, NT], f32, name="t1", tag="t1")
                    nc.vector.tensor_mul(t1[:], src[:, c, :], rep[:, 0:NT])
                    nc.vector.tensor_sub(dst[:, c, :], t1[:], rep[:, NT:2 * NT])

            def kv_half(l, half, h, kb_sb, vb_bc):
                """Compute K^T and V for one feature half and AllGather it."""
                kT = ap_.tile([P, 4, NT], f16, name="kT", tag="hid")
                vT = ap_.tile([P, 2, 512], f16, name="vT", tag="vat")
                for m in range(4 * half, 4 * half + 4):
                    wt = wp.tile([P, CCH, P], f32r, name="wt_k", tag="wA",
                                 bufs=5)
                    nc.sync.dma_start(wt[:], d_wk[l, m])
                    ps = psp.tile([P, 2 * NT], f32, name="ps_k", tag="mm",
                                  bufs=3)
                    for c in range(CCH):
                        nc.tensor.matmul(ps[:, 0:NT], wt[:, c, :], h[:, c, :],
                                         start=(c == 0), stop=(c == CCH - 1))
                    nc.vector.tensor_scalar(kT[:, m - 4 * half, :],
                                            ps[:, 0:NT],
                                            kb_sb[:, m:m + 1], None, ALU.add)
                wv_t = wp.tile([P, CCH, 512], f32r, name="wv_t", tag="wB",
                               bufs=2)
                nc.sync.dma_start(
                    wv_t[:],
                    d_wv[l].rearrange("(c p) f -> p c f", p=P)
                    [:, :, half * 512:(half + 1) * 512])
                for tk in range(2):
                    ps = psp.tile([P, 2 * NT], f32, name="ps_v", tag="mm",
                                  bufs=3)
                    for c in range(CCH):
                        nc.tensor.matmul(
                            ps[:, 0:512], h[:, c, tk * P:(tk + 1) * P],
                            wv_t[:, c, :], start=(c == 0), stop=(c == CCH - 1))
                    nc.vector.tensor_add(
                        vT[:, tk, :], ps[:, 0:512],
                        vb_bc[:, half * 512:(half + 1) * 512])
                kv_in = dp.tile([2, 512 * NT], f16, name="kv_in",
                                tag=f"kv_in{half}")
                kv_out = dp.tile([GROUP, 2, 512 * NT], f16, name="kv_out",
                                 tag=f"kv_out{half}")
                nc.sync.dma_start(
                    kv_in[0].rearrange("(c t) -> c t", t=NT)
                    .rearrange("(c p) t -> p c t", p=P), kT[:])
                nc.sync.dma_start(
                    kv_in[1].rearrange("(g f) -> g f", f=512)
                    .rearrange("(g p) f -> p g f", p=P), vT[:])
                nc.gpsimd.collective_compute(
                    "AllGather", mybir.AluOpType.bypass,
                    replica_groups=groups4,
                    ins=[kv_in.opt()], outs=[kv_out.opt()])
                return kv_out

            # ================= transformer layers =================
            for l in range(L):
                # ---- LN1 ----
                h = ap_.tile([P, CCH, NT], f32r, name="h", tag="h")
                layernorm(x, h)

                qb_sb = smp.tile([P, CCH], f32, name="qb_sb", tag="qb")
                kb_sb = smp.tile([P, CCH], f32, name="kb_sb", tag="kb")
                nc.sync.dma_start(qb_sb[:], d_qb[l])
                nc.sync.dma_start(kb_sb[:], d_kb[l])
                vb_bc = smp.tile([P, C], f32, name="vb_bc", tag="vb")
                vb_l = d_vb[l]
                nc.gpsimd.dma_start(
                    vb_bc[:],
                    bass.AP(tensor=vb_l.tensor, offset=vb_l.offset,
                            ap=[[0, P], *vb_l.ap]))

                # ---- K/V per feature half, AllGather each half ----
                kv_outs = []
                for half in range(2):
                    kv_outs.append(kv_half(l, half, h, kb_sb, vb_bc))

                # ---- Q ----
                qT = ap_.tile([P, CCH, NT], f16, name="qT", tag="qT")
                for m in range(CCH):
                    wt = wp.tile([P, CCH, P], f32r, name="wt_q", tag="wA",
                                 bufs=5)
                    nc.sync.dma_start(wt[:], d_wq[l, m])
                    ps = psp.tile([P, 2 * NT], f32, name="ps_q", tag="mm",
                                  bufs=3)
                    for c in range(CCH):
                        nc.tensor.matmul(ps[:, 0:NT], wt[:, c, :], h[:, c, :],
                                         start=(c == 0), stop=(c == CCH - 1))
                    nc.vector.tensor_scalar(qT[:, m, :], ps[:, 0:NT],
                                            qb_sb[:, m:m + 1], None, ALU.add)

                # ---- attention (16 heads; head pair hp shares K/V tiles) ----
                attn_hm = ap_.tile([HS, H, NT], f32r, name="attn_hm",
                                   tag="hid")
                for hp in range(H // 2):
                    kvo = kv_outs[hp // 4]
                    hpl = hp % 4
                    KT_hp = smp.tile([P, GROUP * NT], f16, name="KT_hp",
                                     tag="KT_hp", bufs=2)
                    for r in range(GROUP):
                        nc.gpsimd.dma_start(
                            KT_hp[:, r * NT:(r + 1) * NT],
                            kvo[r, 0].rearrange("(c t) -> c t", t=NT)
                            [hpl * P:(hpl + 1) * P, :])
                    vaug = smp.tile([P, KB, 2, HS + 1], f16, name="vaug",
                                    tag="vaug", bufs=2)
                    for g in range(KB):
                        r, j = g // 2, g % 2
                        nc.gpsimd.dma_start(
                            vaug[:, g, :, 0:HS],
                            kvo[r, 1].rearrange("(t f) -> t f", f=512)
                            [j * P:(j + 1) * P,
                             hpl * 2 * HS:(hpl + 1) * 2 * HS]
                            .rearrange("p (a b) -> p a b", a=2))
                    nc.vector.tensor_copy(
                        vaug[:, :, :, HS],
                        onr[:, 0:2 * KB].rearrange("p (a b) -> p a b", a=KB))

                    for b in range(2):
                        hh = 2 * hp + b
                        po = psp.tile([P, NT], f32, name="po", tag="po",
                                      bufs=2)
                        for g in range(KB):
                            ps = psp.tile([P, NT], f32, name="ps_s", tag="ps")
                            nc.tensor.matmul(
                                ps[:],
                                KT_hp[b * HS:(b + 1) * HS, g * P:(g + 1) * P],
                                qT[b * HS:(b + 1) * HS, hp, :],
                                start=True, stop=True)
                            sm = smp.tile([P, NT], f32, name="sm_t",
                                          tag="sm_t")
                            nc.vector.tensor_add(sm[:], ps[:], msk[:, g, :])
                            e = smp.tile([P, NT], f16, name="e", tag="e")
                            nc.scalar.activation(e[:], sm[:], AFT.Exp)
                            nc.tensor.matmul(po[0:HS + 1, :], vaug[:, g, b, :],
                                             e[:], start=(g == 0),
                                             stop=(g == KB - 1))
                        rec = smp.tile([P, NT], f32r, name="rec", tag="rec")
                        with nc.allow_low_precision(reason="softmax recip"):
                            nc.vector.reciprocal(rec[HS:HS + 1, :],
                                                 po[HS:HS + 1, :])
                        prep = psp.tile([HS, NT], f32, name="prep", tag="ps")
                        nc.tensor.matmul(prep[:], onr[HS:HS + 1, 0:HS],
                                         rec[HS:HS + 1, :],
                                         start=True, stop=True)
                        rep_s = smp.tile([HS, NT], f32, name="rep_s",
                                         tag="reps")
                        nc.scalar.copy(rep_s[:], prep[:])
                        nc.vector.tensor_mul(attn_hm[:, hh, :], po[0:HS, :],
                                             rep_s[:])

                # reshape heads back to feature-major [P, CCH, NT]
                attn2 = ap_.tile([P, CCH, NT], f32r, name="attn2", tag="vat")
                ahm4 = attn_hm.rearrange("p (m b) t -> p m b t", b=2)
                nc.sync.dma_start(attn2[0:HS, :, :], ahm4[:, :, 0, :])
                nc.sync.dma_start(attn2[HS:P, :, :], ahm4[:, :, 1, :])

                # ---- output projection + residual ----
                bo_sb = smp.tile([1, C], f32r, name="bo_sb", tag="bo")
                nc.sync.dma_start(bo_sb[:], d_bo[l][None, :])
                for m in range(CCH):
                    wt = wp.tile([P, CCH, P], f32r, name="wt_o", tag="wA",
                                 bufs=5)
                    nc.sync.dma_start(wt[:], d_wo[l, m])
                    ps = psp.tile([P, 2 * NT], f32, name="ps_o", tag="mm",
                                  bufs=3)
                    for c in range(CCH):
                        nc.tensor.matmul(ps[:, 0:NT], wt[:, c, :],
                                         attn2[:, c, :],
                                         start=(c == 0), stop=False)
                    nc.tensor.matmul(ps[:, 0:NT],
                                     bo_sb[:, m * P:(m + 1) * P],
                                     onr[0:1, :],
                                     start=False, stop=True)
                    nc.vector.tensor_add(x[:, m, :], ps[:, 0:NT], x[:, m, :])

                # ---- LN2 + FFN ----
                h2 = ap_.tile([P, CCH, NT], f32r, name="h2", tag="h")
                layernorm(x, h2)

                b1_sb = smp.tile([P, FCH], f32, name="b1_sb", tag="qb")
                nc.sync.dma_start(b1_sb[:], d_b1[l])
                hid = ap_.tile([P, FCH, NT], f32r, name="hid", tag="hid")
                for fh in range(FCH):
                    wt = wp.tile([P, CCH, P], f32r, name="wt_1", tag="wA",
                                 bufs=5)
                    nc.sync.dma_start(wt[:], d_w1[l, fh])
                    ps = psp.tile([P, 2 * NT], f32, name="ps_1", tag="mm",
                                  bufs=3)
                    for c in range(CCH):
                        nc.tensor.matmul(ps[:, 0:NT], wt[:, c, :], h2[:, c, :],
                                         start=(c == 0), stop=(c == CCH - 1))
                    nc.vector.tensor_scalar(hid[:, fh, :], ps[:, 0:NT],
                                            b1_sb[:, fh:fh + 1], 0.0,
                                            ALU.add, ALU.max)

                b2_sb = smp.tile([1, C], f32r, name="b2_sb", tag="bo")
                nc.sync.dma_start(b2_sb[:], d_b2[l][None, :])
                for m in range(CCH):
                    wt2 = wp.tile([P, FCH, P], f32r, name="wt_2", tag="wB",
                                  bufs=2)
                    nc.sync.dma_start(wt2[:], d_w2[l, m])
                    ps = psp.tile([P, 2 * NT], f32, name="ps_2", tag="mm",
                                  bufs=3)
                    for fc in range(FCH):
                        nc.tensor.matmul(ps[:, 0:NT], wt2[:, fc, :],
                                         hid[:, fc, :],
                                         start=(fc == 0), stop=False)
                    nc.tensor.matmul(ps[:, 0:NT],
                                     b2_sb[:, m * P:(m + 1) * P],
                                     onr[0:1, :],
                                     start=False, stop=True)
                    nc.vector.tensor_add(x[:, m, :], ps[:, 0:NT], x[:, m, :])

            # ================= final LN + lm_head =================
            zf = ap_.tile([P, CCH, NT], f32r, name="zf", tag="h")
            layernorm(x, zf)

            xf_in = dp.tile([C, NT], f32, name="xf_in", tag="xf_in", bufs=1)
            xf_out = dp.tile([NCORES, C * NT], f32, name="xf_out",
                             tag="xf_out", bufs=1, addr_space="Shared")
            nc.sync.dma_start(
                xf_in.rearrange("(c p) t -> p c t", p=P), zf.bitcast(f32)[:])
            nc.gpsimd.collective_compute(
                "AllGather", mybir.AluOpType.bypass,
                replica_groups=groups8,
                ins=[xf_in.opt()], outs=[xf_out.opt()])

            for n in range(NTK):
                xf_n = wp.tile([P, CCH, TKW], f32r, name="xf_n", tag="wB",
                               bufs=2)
                for r2 in range(2):
                    nc.sync.dma_start(
                        xf_n[:, :, r2 * NT:(r2 + 1) * NT],
                        xf_out[2 * n + r2]
                        .rearrange("(c p t) -> p c t", p=P, t=NT)
                        .bitcast(f32r))
                for m in range(MLM):
                    wlm_t = wp.tile([P, CCH, MV], f32r, name="wlm_t",
                                    tag="wA", bufs=5)
                    nc.sync.dma_start(wlm_t[:], d_wlm[m])
                    ps = psp.tile([P, 2 * NT], f32, name="ps_lm", tag="mm",
                                  bufs=3)
                    for c in range(CCH):
                        nc.tensor.matmul(ps[0:MV, 0:TKW], wlm_t[:, c, :],
                                         xf_n[:, c, :],
                                         start=(c == 0), stop=(c == CCH - 1))
                    lo = smp.tile([P, TKW], f32, name="lo", tag="lo")
                    nc.vector.tensor_scalar(lo[0:MV, :], ps[0:MV, 0:TKW],
                                            blm_sb[0:MV, m:m + 1], None,
                                            ALU.add)
                    nc.sync.dma_start(
                        d_out[m * MV:(m + 1) * MV, n * TKW:(n + 1) * TKW],
                        lo[0:MV, :])

    nc.compile()
    return nc


def _host_prep(inputs):
    """Fold LN scale/bias into weights, pre-tile lhsT weights, build masks."""
    f = np.float32
    g = {k: np.asarray(v) for k, v in inputs.items()}

    tok_emb = g["tok_emb"].astype(f)
    pos_emb = g["pos_emb"].astype(f)
    idx = np.asarray(g["idx"]).astype(np.int64)

    x0 = tok_emb[idx] + pos_emb[None, :T, :]          # [B, T, C]

    def cat_heads(w):                                  # [H, C, HS] -> [C, H*HS]
        return np.ascontiguousarray(w.transpose(1, 0, 2).reshape(C, H * HS))

    def tile_lhst(w, nm):
        # [Cin, Cout] -> [Cout/nm-chunks (m), P(p over Cin), Cin/P (c), f]
        cin = w.shape[0]
        r = w.reshape(cin // P, P, nm, w.shape[1] // nm)   # [c, p, m, f]
        return np.ascontiguousarray(r.transpose(2, 1, 0, 3))  # [m, p, c, f]

    wq = np.empty((L, CCH, P, CCH, P), f)
    wk = np.empty((L, CCH, P, CCH, P), f)
    wo = np.empty((L, CCH, P, CCH, P), f)
    w1 = np.empty((L, FCH, P, CCH, P), f)
    w2 = np.empty((L, CCH, P, FCH, P), f)
    wv = np.empty((L, C, C), f)
    qb = np.empty((L, P, CCH), f)
    kb = np.empty((L, P, CCH), f)
    b1t = np.empty((L, P, FCH), f)
    vb = np.empty((L, C), f)

    scale = 1.0 / np.sqrt(HS)
    for l in range(L):
        s1 = g["ln1_s"][l].astype(f)
        bn1 = g["ln1_b"][l].astype(f)
        s2 = g["ln2_s"][l].astype(f)
        bn2 = g["ln2_b"][l].astype(f)
        Wq = cat_heads(g["Wq"][l].astype(f))
        Wk = cat_heads(g["Wk"][l].astype(f))
        Wv = cat_heads(g["Wv"][l].astype(f))
        wq[l] = tile_lhst(s1[:, None] * Wq * scale, CCH)
        wk[l] = tile_lhst(s1[:, None] * Wk, CCH)
        wo[l] = tile_lhst(g["Wo"][l].astype(f), CCH)
        wv[l] = s1[:, None] * Wv
        qb[l] = ((bn1 @ Wq) * scale).reshape(CCH, P).T
        kb[l] = (bn1 @ Wk).reshape(CCH, P).T
        vb[l] = bn1 @ Wv
        W1 = g["W1"][l].astype(f)
        w1[l] = tile_lhst(s2[:, None] * W1, FCH)
        b1t[l] = (g["b1"][l].astype(f) + bn2 @ W1).reshape(FCH, P).T
        w2[l] = tile_lhst(g["W2"][l].astype(f), CCH)

    sf = g["lnf_s"].astype(f)
    bf = g["lnf_b"].astype(f)
    Wlm = g["W_lm"].astype(f)
    wlm_full = sf[:, None] * Wlm                      # [C, V]
    blm_full = (g["b_lm"].astype(f) + bf @ Wlm)       # [V]

    onr = np.ones((P, NT), f)
    onf = np.ones((P, 1), f)

    shared = dict(
        wq=wq, wk=wk, wo=wo, w1=w1, w2=w2, wv=wv,
        qb=np.ascontiguousarray(qb), kb=np.ascontiguousarray(kb),
        b1=np.ascontiguousarray(b1t),
        bo=g["bo"].astype(f), b2=g["b2"].astype(f), vb=vb,
        onr=onr, onf=onf,
    )

    in_maps = []
    for core in range(NCORES):
        bb, cg = core // GROUP, core % GROUP
        x0t = np.ascontiguousarray(
            x0[bb, cg * NT:(cg + 1) * NT, :].T)       # [C, NT]
        qpos = cg * NT + np.arange(NT)
        kpos = np.arange(T)
        mask = np.where(kpos[:, None] <= qpos[None, :], 0.0, NEG).astype(f)
        mask = np.ascontiguousarray(mask.reshape(KB, P, NT))
        wlm_s = wlm_full[:, core * VSH:(core + 1) * VSH]   # [C, 4000]
        blm_s = blm_full[core * VSH:(core + 1) * VSH]
        blm_t = np.zeros((P, MLM), f)
        blm_t[:MV, :] = blm_s.reshape(MLM, MV).T
        m = dict(shared)
        m["x0t"] = x0t
        m["msk"] = mask
        m["wlm"] = tile_lhst(wlm_s, MLM)              # [32, 128, 8, 125]
        m["blm"] = blm_t
        in_maps.append(m)
    return in_maps


def kernel(**inputs):
    from concourse import bass_utils

    if "nc" not in _CACHE:
        _CACHE["nc"] = _build()
    nc = _CACHE["nc"]

    in_maps = _host_prep(inputs)
    trace = os.environ.get("BIGRAM_TRACE") == "1"
    res = bass_utils.run_bass_kernel_spmd(
        nc, in_maps, core_ids=list(range(NCORES)), trace=trace)
    _CACHE["last_res"] = res

    out = np.empty((B * T, V), np.float32)
    for core in range(NCORES):
        out[:, core * VSH:(core + 1) * VSH] = res.results[core]["logits"].T
    return out.reshape(B, T, V)



# revision 26
# speedup vs baseline: 1.4005x; 1.4005x over previous
"""Trainium2 Bass kernel for the 6-layer BigramLanguageModel (B=2, T=1024,
C=1024, H=16, FFN=4096, V=32000).

Strategy: context-parallel over 8 NeuronCores. Cores 0-3 handle batch 0,
cores 4-7 batch 1. Each core owns query chunks {c, 7-c} (128 tokens each)
of its batch, which balances causal-attention work perfectly across cores
and lets key blocks 4-7 be computed for the hi chunk only (25% less
attention compute), with per-core mask data absorbing all position
dependence so the SPMD program stays uniform.

Per layer each core computes LN/QKV/attention/proj/FFN for its own 256
tokens; communication is a 4-rank AllGather of K/V per layer (fp16, two
feature-halves so attention on the first half overlaps the gather of the
second) plus one final 8-rank AllGather of the pre-lm_head hidden states.
The lm_head is vocab-sharded 8 ways (fp16 weights/outputs); the host
transposes and unpermutes at the end.

All matmul operands are fp16 (fp32 PSUM accumulation); the residual
stream and LN statistics stay fp32. LN scale/bias are folded into the
consuming weights on the host. Softmax skips max-subtraction (scores are
O(1)); exp runs as ONE scalar-engine activation per head over all key
blocks; LN rstd uses a DVE quake-rsqrt so Exp is the only ACT function
(a single table load); softmax denominators use reciprocal_approx_fast;
biases ride fused scalar_tensor_tensor ops instead of K=1 matmuls.
"""

import os

import numpy as np

# model dims (fixed by the problem)
B, T, V, C, H, HS, L, F = 2, 1024, 32000, 1024, 16, 64, 6, 4096
P = 128
NCORES = 8
GROUP = 4            # cores per batch (context-parallel degree)
NT = T // GROUP      # 256 tokens per core (two 128-token chunks)
CCH = C // P         # 8 feature chunks
FCH = F // P         # 32 FFN hidden chunks
KB = T // P          # 8 key blocks
SCW = 4 * NT + 4 * P  # 1536 score/e columns per head (causal-trimmed)
VSH = V // NCORES    # 4000 vocab columns per core
MLM = 32             # lm_head output-partition chunks
MV = VSH // MLM      # 125 vocab rows per chunk
NTK = 4              # lm_head token chunks
TKW = (NCORES * NT) // NTK   # 512 tokens per chunk
EPS = 1e-5
NEG = -30000.0       # mask value; must stay finite in fp16

_CACHE = {}


def _build():
    import concourse.bass as bass
    import concourse.tile as tile
    from concourse import bacc, mybir

    f32 = mybir.dt.float32
    f32r = mybir.dt.float32r
    f16 = mybir.dt.float16
    i32 = mybir.dt.int32
    AFT = mybir.ActivationFunctionType
    ALU = mybir.AluOpType
    MAGIC = 0x5F375A86          # quake rsqrt seed constant
    MAGICR = 0x7EF311C3         # quake reciprocal seed constant

    nc = bacc.Bacc("TRN2", target_bir_lowering=False, debug=False,
                   num_devices=NCORES)

    def din(name, shape, dt=f16):
        return nc.dram_tensor(name, shape, dt, kind="ExternalInput").ap()

    d_x0 = din("x0t", [C, NT], f32r)
    d_wq = din("wq", [L, CCH, P, CCH, P])
    d_wk = din("wk", [L, CCH, P, CCH, P])
    d_wo = din("wo", [L, CCH, P, CCH, P])
    d_w1 = din("w1", [L, FCH, P, CCH, P])
    d_w2 = din("w2", [L, CCH, P, FCH, P])
    d_wv = din("wv", [L, C, C])
    d_wlm = din("wlm", [MLM, P, CCH, MV])
    d_qb = din("qb", [L, P, CCH], f32)
    d_kb = din("kb", [L, P, CCH], f32)
    d_b1 = din("b1", [L, P, FCH], f32)
    d_bo = din("bo", [L, P, CCH], f32)
    d_b2 = din("b2", [L, P, CCH], f32)
    d_vb = din("vb", [L, C], f32)
    d_blm = din("blm", [P, MLM], f32)
    d_msk = din("msk", [P, SCW], f32)
    d_onr = din("onr", [P, NT], f32r)
    d_onf = din("onf", [P, 1], f32r)

    d_out = nc.dram_tensor("logits", [VSH, NCORES * NT], f16,
                           kind="ExternalOutput").ap()

    groups4 = [[0, 1, 2, 3], [4, 5, 6, 7]]
    groups8 = [list(range(NCORES))]

    # e/score column offset for key block g (blocks 0-3: both query
    # chunks, N=256; blocks 4-7: hi chunk only, N=128)
    def gcol(g):
        return g * NT if g < 4 else 4 * NT + (g - 4) * P

    def gwid(g):
        return NT if g < 4 else P

    with tile.TileContext(nc) as tc:
        with tc.tile_pool(name="persist", bufs=1) as pp, \
             tc.tile_pool(name="act", bufs=1) as ap_, \
             tc.tile_pool(name="wp", bufs=3) as wp, \
             tc.tile_pool(name="sm", bufs=2) as smp, \
             tc.tile_pool(name="ps", bufs=2, space="PSUM") as psp, \
             tc.tile_pool(name="dram", bufs=2, space="DRAM") as dp:

            # ---------------- persistent tiles ----------------
            x = pp.tile([P, CCH, NT], f32r)         # residual stream
            msk = pp.tile([P, SCW], f32)            # causal mask (data)
            onr = pp.tile([P, NT], f32r)
            onf = pp.tile([P, 1], f32r)
            eps_t = pp.tile([1, 1], f32)
            magic_t = pp.tile([1, NT], i32)
            magicr_t = pp.tile([1, NT], i32)
            blm_sb = pp.tile([P, MLM], f32)
            KTf = pp.tile([P, CCH, T], f16)         # K^T staged, all heads
            vaug = pp.tile([P, KB, H, HS + 1], f16)  # V + ones col
            on16 = pp.tile([P, P], f16)

            nc.sync.dma_start(x[:], d_x0.rearrange("(c p) t -> p c t", p=P))
            nc.sync.dma_start(msk[:], d_msk[:])
            nc.sync.dma_start(onr[:], d_onr[:])
            nc.sync.dma_start(onf[:], d_onf[:])
            nc.sync.dma_start(blm_sb[:], d_blm[:])
            nc.vector.memset(eps_t[:], EPS)
            nc.vector.memset(magic_t[:], MAGIC)
            nc.vector.memset(magicr_t[:], MAGICR)
            nc.vector.memset(on16[:], 1.0)
            # ones column of vaug, written once (DMAs only touch 0:HS)
            nc.vector.tensor_copy(
                vaug[:, :, :, HS],
                on16[:, 0:KB * H].rearrange("p (a b) -> p a b", a=KB))

            def layernorm(src, dst):
                """dst (f16) = (src - mu) / sqrt(var + eps), feature-major.

                All stats run on the vector engine (square via multiply,
                rstd via quake-rsqrt + 2 Newton steps) so Exp stays the
                kernel's only ACT function — a single table set."""
                # the two stat chains MUST live in different PSUM banks:
                # start=True clears has_written for the whole bank, so two
                # interleaved accumulation chains in one bank corrupt each
                # other (the first chain loses its first term)
                ps_sx = psp.tile([1, NT], f32, name="ps_sx", tag="pr",
                                 bufs=1)
                ps_sq = psp.tile([1, NT], f32, name="ps_sq", tag="po",
                                 bufs=2)
                sq = smp.tile([P, CCH, NT], f32r, name="sq", tag="sq",
                               bufs=1)
                nc.vector.tensor_mul(sq[:], src[:], src[:])
                for c in range(CCH):
                    nc.tensor.matmul(ps_sx[:], onf[:], src[:, c, :],
                                     start=(c == 0), stop=(c == CCH - 1))
                    nc.tensor.matmul(ps_sq[:], onf[:], sq[:, c, :],
                                     start=(c == 0), stop=(c == CCH - 1))
                st = smp.tile([1, 6, NT], f32, name="st_s", tag="st_s")
                sti = st.bitcast(i32)
                mu = st[:, 0, :]
                musq = st[:, 1, :]
                var = st[:, 2, :]
                y = st[:, 3, :]
                t = st[:, 4, :]
                s = st[:, 5, :]
                nc.vector.tensor_scalar(mu, ps_sx[:], 1.0 / C, None,
                                        ALU.mult)
                nc.vector.tensor_mul(musq, mu, mu)
                nc.vector.scalar_tensor_tensor(var, ps_sq[:], 1.0 / C,
                                               musq, ALU.mult, ALU.subtract)
                nc.vector.tensor_scalar(var, var, EPS, None, ALU.add)
                # y0 = quake-rsqrt seed: MAGIC - (bits(var) >> 1)
                nc.vector.tensor_scalar(sti[:, 4, :], sti[:, 2, :], 1, None,
                                        ALU.logical_shift_right)
                nc.vector.tensor_sub(sti[:, 3, :], magic_t[:], sti[:, 4, :])
                rsrc = smp.tile([1, 2 * NT], f32r, name="rsrc", tag="rsrc")
                with nc.allow_low_precision(reason="LN rstd in f32r"):
                    for it in range(2):
                        nc.vector.tensor_mul(t, var, y)
                        nc.vector.tensor_mul(t, t, y)
                        nc.vector.tensor_scalar(s, t, -0.5, 1.5,
                                                ALU.mult, ALU.add)
                        if it == 0:
                            nc.vector.tensor_mul(y, y, s)
                        else:
                            nc.vector.tensor_mul(rsrc[:, 0:NT], y, s)
                    nc.vector.tensor_mul(rsrc[:, NT:2 * NT], mu,
                                         rsrc[:, 0:NT])
                ps_rep = psp.tile([P, 2 * NT], f32, name="ps_rep", tag="mm",
                                  bufs=5)
                nc.tensor.matmul(ps_rep[:], onr[0:1, 0:P], rsrc[:],
                                 start=True, stop=True)
                rep = smp.tile([P, 2 * NT], f32, name="rep", tag="rep")
                nc.vector.tensor_copy(rep[:], ps_rep[:])
                for c in range(CCH):
                    t1 = smp.tile([P, NT], f32, name="t1", tag="t1")
                    nc.vector.tensor_mul(t1[:], src[:, c, :], rep[:, 0:NT])
                    nc.vector.tensor_sub(dst[:, c, :], t1[:], rep[:, NT:2 * NT])

            def kv_half(l, half, h, kb_sb, vb_bc):
                """Compute K^T and V for one feature half and AllGather it."""
                kT = ap_.tile([P, 4, NT], f16, name="kT", tag="vat")
                vT = ap_.tile([P, 2, 512], f16, name="vT", tag="vat2")
                for m in range(4 * half, 4 * half + 4):
                    wt = wp.tile([P, CCH, P], f16, name="wt_k", tag="wA",
                                 bufs=5)
                    nc.sync.dma_start(wt[:], d_wk[l, m])
                    ps = psp.tile([P, 2 * NT], f32, name="ps_k", tag="mm",
                                  bufs=5)
                    for c in range(CCH):
                        nc.tensor.matmul(ps[:, 0:NT], wt[:, c, :], h[:, c, :],
                                         start=(c == 0), stop=(c == CCH - 1))
                    nc.vector.tensor_scalar(kT[:, m - 4 * half, :],
                                            ps[:, 0:NT],
                                            kb_sb[:, m:m + 1], None, ALU.add)
                wv_t = wp.tile([P, CCH, 512], f16, name="wv_t", tag="wB",
                               bufs=2)
                nc.sync.dma_start(
                    wv_t[:],
                    d_wv[l].rearrange("(c p) f -> p c f", p=P)
                    [:, :, half * 512:(half + 1) * 512])
                for tk in range(2):
                    ps = psp.tile([P, 2 * NT], f32, name="ps_v", tag="mm",
                                  bufs=5)
                    for c in range(CCH):
                        nc.tensor.matmul(
                            ps[:, 0:512], h[:, c, tk * P:(tk + 1) * P],
                            wv_t[:, c, :], start=(c == 0), stop=(c == CCH - 1))
                    nc.vector.tensor_add(
                        vT[:, tk, :], ps[:, 0:512],
                        vb_bc[:, half * 512:(half + 1) * 512])
                kv_in = dp.tile([2, 512 * NT], f16, name="kv_in",
                                tag=f"kv_in{half}")
                kv_out = dp.tile([GROUP, 2, 512 * NT], f16, name="kv_out",
                                 tag=f"kv_out{half}")
                nc.scalar.dma_start(
                    kv_in[0].rearrange("(c t) -> c t", t=NT)
                    .rearrange("(c p) t -> p c t", p=P), kT[:])
                nc.scalar.dma_start(
                    kv_in[1].rearrange("(g f) -> g f", f=512)
                    .rearrange("(g p) f -> p g f", p=P), vT[:])
                nc.gpsimd.collective_compute(
                    "AllGather", mybir.AluOpType.bypass,
                    replica_groups=groups4,
                    ins=[kv_in.opt()], outs=[kv_out.opt()])
                return kv_out

            def stage_half(half, kvo):
                """Gather one feature-half of the AllGathered K/V into the
                persistent SBUF staging tiles (KTf / vaug)."""
                for r in range(GROUP):
                    # K^T: rank r supplies key blocks r (lo) and 7-r (hi)
                    src = kvo[r, 0].rearrange("(c t) -> c t", t=NT) \
                        .rearrange("(c p) t -> p c t", p=P)
                    for j, g in ((0, r), (1, 7 - r)):
                        nc.scalar.dma_start(
                            KTf[:, half * 4:(half + 1) * 4,
                                g * P:(g + 1) * P],
                            src[:, :, j * P:(j + 1) * P])
                    vsrc = kvo[r, 1].rearrange("(t f) -> t f", f=512)
                    for j, g in ((0, r), (1, 7 - r)):
                        nc.scalar.dma_start(
                            vaug[:, g, half * 8:(half + 1) * 8, 0:HS],
                            vsrc[j * P:(j + 1) * P, :]
                            .rearrange("p (m f) -> p m f", m=8))

            # ================= transformer layers =================
            for l in range(L):
                # ---- LN1 ----
                h = ap_.tile([P, CCH, NT], f16, name="h", tag="h")
                layernorm(x, h)

                qb_sb = smp.tile([P, CCH], f32, name="qb_sb", tag="qb")
                kb_sb = smp.tile([P, CCH], f32, name="kb_sb", tag="kb")
                bo_sb = smp.tile([P, CCH], f32, name="bo_sb", tag="bo")
                b2_sb = smp.tile([P, CCH], f32, name="b2_sb", tag="b2")
                nc.sync.dma_start(qb_sb[:], d_qb[l])
                nc.sync.dma_start(kb_sb[:], d_kb[l])
                nc.sync.dma_start(bo_sb[:], d_bo[l])
                nc.sync.dma_start(b2_sb[:], d_b2[l])
                vb_bc = smp.tile([P, C], f32, name="vb_bc", tag="vb",
                                 bufs=1)
                vb_l = d_vb[l]
                nc.gpsimd.dma_start(
                    vb_bc[:],
                    bass.AP(tensor=vb_l.tensor, offset=vb_l.offset,
                            ap=[[0, P], *vb_l.ap]))

                # ---- K/V per feature half, AllGather each half ----
                kv_outs = []
                for half in range(2):
                    kv_outs.append(kv_half(l, half, h, kb_sb, vb_bc))

                # ---- Q ----
                qT = ap_.tile([P, CCH, NT], f16, name="qT", tag="qT")
                for m in range(CCH):
                    wt = wp.tile([P, CCH, P], f16, name="wt_q", tag="wA",
                                 bufs=5)
                    nc.sync.dma_start(wt[:], d_wq[l, m])
                    ps = psp.tile([P, 2 * NT], f32, name="ps_q", tag="mm",
                                  bufs=5)
                    for c in range(CCH):
                        nc.tensor.matmul(ps[:, 0:NT], wt[:, c, :], h[:, c, :],
                                         start=(c == 0), stop=(c == CCH - 1))
                    nc.vector.tensor_scalar(qT[:, m, :], ps[:, 0:NT],
                                            qb_sb[:, m:m + 1], None, ALU.add)

                # ---- stage K/V halves into SBUF as they arrive ----
                stage_half(0, kv_outs[0])
                stage_half(1, kv_outs[1])

                # ---- attention ----
                attn_hm = ap_.tile([HS, H, NT], f16, name="attn_hm",
                                   tag="big")
                for hp in range(CCH):
                    half, hpl = hp // 4, hp % 4
                    cch = half * 4 + hpl          # KTf chunk for this pair
                    # scores for both heads of the pair, emitted
                    # interleaved so the K=64 matmuls row-pack in the PE
                    pss = []
                    for b in range(2):
                        psA0 = psp.tile([P, 2 * NT], f32, name="psA0",
                                        tag="mm", bufs=5)
                        psA1 = psp.tile([P, 2 * NT], f32, name="psA1",
                                        tag="mm", bufs=5)
                        psB = psp.tile([P, 2 * NT], f32, name="psB",
                                       tag="mm", bufs=5)
                        pss.append((psA0, psA1, psB))
                    for g in range(KB):
                        for b in range(2):
                            psA0, psA1, psB = pss[b]
                            dst = (psA0 if g < 2 else
                                   psA1 if g < 4 else psB)
                            off = (g % 2) * NT if g < 4 else (g - 4) * P
                            qs = (qT[b * HS:(b + 1) * HS, hp, :] if g < 4 else
                                  qT[b * HS:(b + 1) * HS, hp, P:2 * P])
                            nc.tensor.matmul(
                                dst[:, off:off + gwid(g)],
                                KTf[b * HS:(b + 1) * HS, cch,
                                    g * P:(g + 1) * P],
                                qs, start=True, stop=True)
                    for b in range(2):
                        hh = 2 * hp + b
                        psA0, psA1, psB = pss[b]
                        sm = smp.tile([P, SCW], f32, name="sm_t", tag="sm_t")
                        nc.vector.tensor_add(sm[:, 0:512], psA0[:],
                                             msk[:, 0:512])
                        nc.vector.tensor_add(sm[:, 512:1024], psA1[:],
                                             msk[:, 512:1024])
                        nc.vector.tensor_add(sm[:, 1024:1536], psB[:],
                                             msk[:, 1024:1536])
                        e = smp.tile([P, SCW], f16, name="e", tag="e")
                        nc.scalar.activation(e[:], sm[:], AFT.Exp)
                        po = psp.tile([P, NT], f32, name="po", tag="po",
                                      bufs=2)
                        for g in range(KB):
                            nc.tensor.matmul(
                                po[0:HS + 1,
                                   0:NT] if g < 4 else po[0:HS + 1, P:2 * P],
                                vaug[:, g, hh, :],
                                e[:, gcol(g):gcol(g) + gwid(g)],
                                start=(g == 0), stop=(g == KB - 1))
                        # rec = 1/den via bit-trick seed + 2 Newton steps
                        # (reciprocal_approx_fast is unsupported by this
                        # walrus; nc.vector.reciprocal costs 8 cyc/elem)
                        rscr = smp.tile([1, 2, NT], f32, name="rscr",
                                        tag="rec2")
                        rsi = rscr.bitcast(i32)
                        rec = smp.tile([1, NT], f32r, name="rec", tag="rec")
                        den = po[HS:HS + 1, :]
                        with nc.allow_low_precision(reason="softmax recip"):
                            nc.vector.tensor_sub(rsi[:, 0, :], magicr_t[:],
                                                 po.bitcast(i32)[HS:HS + 1, :])
                            nc.vector.tensor_mul(rscr[:, 1, :], den,
                                                 rscr[:, 0, :])
                            nc.vector.tensor_scalar(rscr[:, 1, :],
                                                    rscr[:, 1, :], -1.0, 2.0,
                                                    ALU.mult, ALU.add)
                            nc.vector.tensor_mul(rscr[:, 0, :],
                                                 rscr[:, 0, :], rscr[:, 1, :])
                            nc.vector.tensor_mul(rscr[:, 1, :], den,
                                                 rscr[:, 0, :])
                            nc.vector.tensor_scalar(rscr[:, 1, :],
                                                    rscr[:, 1, :], -1.0, 2.0,
                                                    ALU.mult, ALU.add)
                            nc.vector.tensor_mul(rec[:], rscr[:, 0, :],
                                                 rscr[:, 1, :])
                        prep = psp.tile([HS, NT], f32, name="prep", tag="pr",
                                        bufs=1)
                        nc.tensor.matmul(prep[:], onr[0:1, 0:HS], rec[:],
                                         start=True, stop=True)
                        rep_s = smp.tile([HS, NT], f32, name="rep_s",
                                         tag="reps")
                        nc.vector.tensor_copy(rep_s[:], prep[:])
                        nc.vector.tensor_mul(attn_hm[:, hh, :], po[0:HS, :],
                                             rep_s[:])

                # reshape heads back to feature-major [P, CCH, NT]
                attn2 = ap_.tile([P, CCH, NT], f16, name="attn2", tag="vat")
                ahm4 = attn_hm.rearrange("p (m b) t -> p m b t", b=2)
                nc.scalar.dma_start(attn2[0:HS, :, :], ahm4[:, :, 0, :])
                nc.scalar.dma_start(attn2[HS:P, :, :], ahm4[:, :, 1, :])

                # ---- output projection + residual ----
                for m in range(CCH):
                    wt = wp.tile([P, CCH, P], f16, name="wt_o", tag="wA",
                                 bufs=5)
                    nc.sync.dma_start(wt[:], d_wo[l, m])
                    ps = psp.tile([P, 2 * NT], f32, name="ps_o", tag="mm",
                                  bufs=5)
                    for c in range(CCH):
                        nc.tensor.matmul(ps[:, 0:NT], wt[:, c, :],
                                         attn2[:, c, :],
                                         start=(c == 0), stop=(c == CCH - 1))
                    nc.vector.scalar_tensor_tensor(
                        x[:, m, :], ps[:, 0:NT], bo_sb[:, m:m + 1],
                        x[:, m, :], ALU.add, ALU.add)

                # ---- LN2 + FFN ----
                h2 = ap_.tile([P, CCH, NT], f16, name="h2", tag="h")
                layernorm(x, h2)

                b1_sb = smp.tile([P, FCH], f32, name="b1_sb", tag="qb")
                nc.sync.dma_start(b1_sb[:], d_b1[l])
                hid = ap_.tile([P, FCH, NT], f16, name="hid", tag="big")
                for fh in range(FCH):
                    wt = wp.tile([P, CCH, P], f16, name="wt_1", tag="wA",
                                 bufs=5)
                    nc.sync.dma_start(wt[:], d_w1[l, fh])
                    ps = psp.tile([P, 2 * NT], f32, name="ps_1", tag="mm",
                                  bufs=5)
                    for c in range(CCH):
                        nc.tensor.matmul(ps[:, 0:NT], wt[:, c, :], h2[:, c, :],
                                         start=(c == 0), stop=(c == CCH - 1))
                    nc.vector.tensor_scalar(hid[:, fh, :], ps[:, 0:NT],
                                            b1_sb[:, fh:fh + 1], 0.0,
                                            ALU.add, ALU.max)

                for m in range(CCH):
                    wt2 = wp.tile([P, FCH, P], f16, name="wt_2", tag="wB",
                                  bufs=2)
                    nc.sync.dma_start(wt2[:], d_w2[l, m])
                    ps = psp.tile([P, 2 * NT], f32, name="ps_2", tag="mm",
                                  bufs=5)
                    for fc in range(FCH):
                        nc.tensor.matmul(ps[:, 0:NT], wt2[:, fc, :],
                                         hid[:, fc, :],
                                         start=(fc == 0), stop=(fc == FCH - 1))
                    nc.vector.scalar_tensor_tensor(
                        x[:, m, :], ps[:, 0:NT], b2_sb[:, m:m + 1],
                        x[:, m, :], ALU.add, ALU.add)

            # ================= final LN + lm_head =================
            zf = ap_.tile([P, CCH, NT], f16, name="zf", tag="h")
            layernorm(x, zf)

            # gather the final hidden states in two feature-halves so the
            # first half of each lm_head chain starts while the second
            # half is still gathering
            xf_full = ap_.tile([P, CCH, NCORES * NT], f16, name="xf_full",
                               tag="big")
            for hf in range(2):
                xf_in = dp.tile([C // 2, NT], f16, name=f"xf_in{hf}",
                                tag=f"xf_in{hf}", bufs=1)
                xf_out = dp.tile([NCORES, (C // 2) * NT], f16,
                                 name=f"xf_out{hf}", tag=f"xf_out{hf}",
                                 bufs=1, addr_space="Shared")
                nc.scalar.dma_start(
                    xf_in.rearrange("(c p) t -> p c t", p=P),
                    zf[:, 4 * hf:4 * (hf + 1), :])
                nc.gpsimd.collective_compute(
                    "AllGather", mybir.AluOpType.bypass,
                    replica_groups=groups8,
                    ins=[xf_in.opt()], outs=[xf_out.opt()])
                for r2 in range(NCORES):
                    nc.scalar.dma_start(
                        xf_full[:, 4 * hf:4 * (hf + 1),
                                r2 * NT:(r2 + 1) * NT],
                        xf_out[r2].rearrange("(c p t) -> p c t", p=P, t=NT))
            for m in range(MLM):
                wlm_t = wp.tile([P, CCH, MV], f16, name="wlm_t",
                                tag="wA", bufs=5)
                nc.sync.dma_start(wlm_t[:], d_wlm[m])
                for n in range(NTK):
                    ps = psp.tile([P, 2 * NT], f32, name="ps_lm", tag="mm",
                                  bufs=5)
                    for c in range(CCH):
                        nc.tensor.matmul(
                            ps[0:MV, 0:TKW], wlm_t[:, c, :],
                            xf_full[:, c, n * TKW:(n + 1) * TKW],
                            start=(c == 0), stop=(c == CCH - 1))
                    lo = smp.tile([P, TKW], f16, name="lo", tag="lo")
                    nc.vector.tensor_scalar(lo[0:MV, :], ps[0:MV, 0:TKW],
                                            blm_sb[0:MV, m:m + 1], None,
                                            ALU.add)
                    nc.sync.dma_start(
                        d_out[m * MV:(m + 1) * MV, n * TKW:(n + 1) * TKW],
                        lo[0:MV, :])

    nc.compile()
    return nc


def _host_prep(inputs):
    """Fold LN scale/bias into weights, pre-tile lhsT weights, build masks."""
    f = np.float32
    h16 = np.float16
    g = {k: np.asarray(v) for k, v in inputs.items()}

    tok_emb = g["tok_emb"].astype(f)
    pos_emb = g["pos_emb"].astype(f)
    idx = np.asarray(g["idx"]).astype(np.int64)

    x0 = tok_emb[idx] + pos_emb[None, :T, :]          # [B, T, C]

    def cat_heads(w):                                  # [H, C, HS] -> [C, H*HS]
        return np.ascontiguousarray(w.transpose(1, 0, 2).reshape(C, H * HS))

    def tile_lhst(w, nm):
        # [Cin, Cout] -> [Cout/nm-chunks (m), P(p over Cin), Cin/P (c), f]
        cin = w.shape[0]
        r = w.reshape(cin // P, P, nm, w.shape[1] // nm)   # [c, p, m, f]
        return np.ascontiguousarray(
            r.transpose(2, 1, 0, 3).astype(h16))           # [m, p, c, f]

    wq = np.empty((L, CCH, P, CCH, P), h16)
    wk = np.empty((L, CCH, P, CCH, P), h16)
    wo = np.empty((L, CCH, P, CCH, P), h16)
    w1 = np.empty((L, FCH, P, CCH, P), h16)
    w2 = np.empty((L, CCH, P, FCH, P), h16)
    wv = np.empty((L, C, C), h16)
    qb = np.empty((L, P, CCH), f)
    kb = np.empty((L, P, CCH), f)
    b1t = np.empty((L, P, FCH), f)
    bo_t = np.empty((L, P, CCH), f)
    b2t = np.empty((L, P, CCH), f)
    vb = np.empty((L, C), f)

    scale = 1.0 / np.sqrt(HS)
    for l in range(L):
        s1 = g["ln1_s"][l].astype(f)
        bn1 = g["ln1_b"][l].astype(f)
        s2 = g["ln2_s"][l].astype(f)
        bn2 = g["ln2_b"][l].astype(f)
        Wq = cat_heads(g["Wq"][l].astype(f))
        Wk = cat_heads(g["Wk"][l].astype(f))
        Wv = cat_heads(g["Wv"][l].astype(f))
        wq[l] = tile_lhst(s1[:, None] * Wq * scale, CCH)
        wk[l] = tile_lhst(s1[:, None] * Wk, CCH)
        wo[l] = tile_lhst(g["Wo"][l].astype(f), CCH)
        wv[l] = (s1[:, None] * Wv).astype(h16)
        qb[l] = ((bn1 @ Wq) * scale).reshape(CCH, P).T
        kb[l] = (bn1 @ Wk).reshape(CCH, P).T
        vb[l] = bn1 @ Wv
        bo_t[l] = g["bo"][l].astype(f).reshape(CCH, P).T
        b2t[l] = g["b2"][l].astype(f).reshape(CCH, P).T
        W1 = g["W1"][l].astype(f)
        w1[l] = tile_lhst(s2[:, None] * W1, FCH)
        b1t[l] = (g["b1"][l].astype(f) + bn2 @ W1).reshape(FCH, P).T
        w2[l] = tile_lhst(g["W2"][l].astype(f), CCH)

    sf = g["lnf_s"].astype(f)
    bf = g["lnf_b"].astype(f)
    Wlm = g["W_lm"].astype(f)
    wlm_full = sf[:, None] * Wlm                      # [C, V]
    blm_full = (g["b_lm"].astype(f) + bf @ Wlm)       # [V]

    onr = np.ones((P, NT), f)
    onf = np.ones((P, 1), f)

    shared = dict(
        wq=wq, wk=wk, wo=wo, w1=w1, w2=w2, wv=wv,
        qb=np.ascontiguousarray(qb), kb=np.ascontiguousarray(kb),
        b1=np.ascontiguousarray(b1t),
        bo=np.ascontiguousarray(bo_t), b2=np.ascontiguousarray(b2t),
        vb=vb, onr=onr, onf=onf,
    )

    in_maps = []
    for core in range(NCORES):
        bb, cg = core // GROUP, core % GROUP
        tsel = np.concatenate([np.arange(cg * P, (cg + 1) * P),
                               np.arange((7 - cg) * P, (8 - cg) * P)])
        x0t = np.ascontiguousarray(x0[bb, tsel, :].T)  # [C, NT]
        qpos = tsel                                    # global query positions
        mask = np.empty((P, SCW), f)
        for gblk in range(KB):
            kpos = gblk * P + np.arange(P)
            off = gcol_host(gblk)
            qp = qpos if gblk < 4 else qpos[P:]
            mask[:, off:off + len(qp)] = np.where(
                kpos[:, None] <= qp[None, :], 0.0, NEG)
        wlm_s = wlm_full[:, core * VSH:(core + 1) * VSH]   # [C, 4000]
        blm_s = blm_full[core * VSH:(core + 1) * VSH]
        blm_t = np.zeros((P, MLM), f)
        blm_t[:MV, :] = blm_s.reshape(MLM, MV).T
        m = dict(shared)
        m["x0t"] = x0t
        m["msk"] = np.ascontiguousarray(mask)
        m["wlm"] = tile_lhst(wlm_s, MLM)              # [32, 128, 8, 125]
        m["blm"] = blm_t
        in_maps.append(m)
    return in_maps


def gcol_host(g):
    return g * NT if g < 4 else 4 * NT + (g - 4) * P


def kernel(**inputs):
    from concourse import bass_utils

    if "nc" not in _CACHE:
        _CACHE["nc"] = _build()
    nc = _CACHE["nc"]

    in_maps = _host_prep(inputs)
    trace = os.environ.get("BIGRAM_TRACE") == "1"
    res = bass_utils.run_bass_kernel_spmd(
        nc, in_maps, core_ids=list(range(NCORES)), trace=trace)
    _CACHE["last_res"] = res

    # device logits: [VSH, 8*NT] f16 per core, token cols in per-core
    # permuted order; assemble + unpermute + cast
    full = np.concatenate(
        [res.results[core]["logits"] for core in range(NCORES)],
        axis=0)                                       # [V, 8*NT]
    tok_global = np.empty(NCORES * NT, np.int64)
    for core in range(NCORES):
        bb, cg = core // GROUP, core % GROUP
        tsel = np.concatenate([np.arange(cg * P, (cg + 1) * P),
                               np.arange((7 - cg) * P, (8 - cg) * P)])
        tok_global[core * NT:(core + 1) * NT] = bb * T + tsel
    out = np.empty((B * T, V), np.float32)
    out[tok_global, :] = full.T.astype(np.float32)
    return out.reshape(B, T, V)
